# revision 16
# baseline (speedup 1.0000x reference)
"""CARAFE content-aware upsampling on 8 Trainium2 NeuronCores (Bass/Tile).

Problem: features (4,128,64,64) f32, masks (4,25,128,128) f32
         -> out (4,128,128,128) f32
out[n,c,2h+a,2w+b] = sum_{i,j in 5x5} f[n,c,h+i-2,w+j-2] * m[n,5i+j,2h+a,2w+b]

v6 strategy (per core = one (n, h-half) shard), all-bf16 on device:
  Weight-stationary over shard feature rows r (36 of them): one bf16
  LDWEIGHTS of fT_row(r) [w''(68), c(128)] feeds up to 5 accumulating
  matmuls, one per in-flight job h = r-4..r (i = r-h), each into its own
  PSUM tile:  out_job(h) [c, (a,wup)=256] += fT_row(h+i).T @ B_i(h).
  B_i is the banded mask matrix [68, 256-in-288-region], materialized per
  job by an SBUF->SBUF diagonal-scatter DMA (340 packets of 40 B) into
  one of 12 rotating per-job band buffers.  Each HWDGE ring drains ~0.235
  packets/ns, so per-job scatters alternate sync/scalar (~1.45 us each,
  ~23 us/queue total, hidden under the ~34 us matmul stream); scatters
  are emitted 6 rows ahead so their WAR waits are pre-satisfied.  Band
  zeros are memset once (f32-bitcast halves the element count) split
  across vector+gpsimd; big-packet loads (features, late masks) ride
  gpsimd's 16-engine SWDGE queue.  PSUM f32 results are cast-copied to
  bf16 staging (vector/scalar alternating) and DMA'd out 8 jobs per
  store on gpsimd; the host upcasts to f32.  rel err ~3e-3 vs 2e-2 gate.
"""
import sys

if "/opt/trn_rl_repo" not in sys.path:
    sys.path.insert(0, "/opt/trn_rl_repo")

from contextlib import ExitStack

import ml_dtypes
import numpy as np

import concourse.tile as tile
from concourse import bacc, mybir
from concourse.ap import AP
from concourse.bass_utils import run_bass_kernel_spmd

# ---- problem constants (hardcoded per harness contract) ----
N = 4
C = 128
H = 64
W = 64
KS = 5
PAD = 2
SCALE = 2
WP = W + KS - 1          # 68 contraction width per feature row
NB = SCALE * W           # 128 upsampled cols per hup row
RUN = 4 * KS             # 20 elems per diagonal run (w,b,a interleaved)
REG = 2 * NB + 32        # 288 per-band region: 16 pad | 256 data | 16 pad
BW = KS * REG            # 1440 band elems per job
NH = H // 2              # 32 low-res rows (jobs) per core
NROWS = NH + 4           # 36 feature rows per shard (halo zero-padded)
NBUF = 12                # rotating per-job band buffers
MSK_COLS = NH * KS * RUN

F32 = mybir.dt.float32
BF16 = mybir.dt.bfloat16

_PROG_CACHE: dict = {}


def _device_body(tc, ctx, out_ap, ft_ap, msk3_ap):
    nc = tc.nc
    sb = ctx.enter_context(tc.tile_pool(name="sb", bufs=1))
    psum = ctx.enter_context(tc.tile_pool(name="ps", bufs=8, space="PSUM"))
    obp = ctx.enter_context(tc.tile_pool(name="ob", bufs=2))

    ft = sb.tile([WP, NROWS * C], BF16)
    mst = sb.tile([WP, MSK_COLS], BF16)
    bufs = [
        sb.tile([WP, BW], BF16, name=f"bb{k}", tag=f"bb{k}") for k in range(NBUF)
    ]

    # all loads ride gpsimd's SWDGE queue, which spreads one dma_start's
    # descriptors across 16 DMA engines (each HWDGE dma_start drains on a
    # single engine); band zero-fill (f32-bitcast) split vector/gpsimd
    nc.gpsimd.memset(bufs[1][:].bitcast(F32), 0.0)
    nc.gpsimd.dma_start(mst[:, : 8 * KS * RUN], msk3_ap[:, : 8 * KS * RUN])
    for k in range(0, NBUF, 2):
        nc.vector.memset(bufs[k][:].bitcast(F32), 0.0)
    nc.gpsimd.dma_start(ft[:, : 6 * C], ft_ap[:, : 6 * C])
    nc.gpsimd.dma_start(mst[:, 8 * KS * RUN :], msk3_ap[:, 8 * KS * RUN :])
    nc.gpsimd.memset(bufs[3][:].bitcast(F32), 0.0)
    nc.gpsimd.memset(bufs[5][:].bitcast(F32), 0.0)
    nc.gpsimd.dma_start(ft[:, 6 * C : 18 * C], ft_ap[:, 6 * C : 18 * C])
    for k in (7, 9, 11):
        nc.gpsimd.memset(bufs[k][:].bitcast(F32), 0.0)
    nc.gpsimd.dma_start(ft[:, 18 * C :], ft_ap[:, 18 * C :])

    def scatter(j):
        # job j's 5 band regions: 340 diagonal 40 B runs, alternating rings
        g = bufs[j % NBUF][:]
        dst = AP(g.tensor, g.offset, [[BW + 4, WP], [REG, KS], [1, RUN]])
        m = mst[:]
        src = AP(
            m.tensor,
            m.offset + j * KS * RUN,
            [[MSK_COLS, WP], [RUN, KS], [1, RUN]],
        )
        eng = nc.sync if j % 2 == 0 else nc.scalar
        eng.dma_start(dst, src)

    for j in range(8):
        scatter(j)

    pt = {}
    ob = None
    for r in range(NROWS):
        lhsT = ft[:, r * C : (r + 1) * C]
        for h in range(max(0, r - 4), min(NH - 1, r) + 1):
            i = r - h
            if i == 0:
                pt[h] = psum.tile([C, 2 * NB], F32, name=f"pt{h}", tag="pt")
            g = bufs[h % NBUF][:]
            # psum col = a*NB + wup (a outer, (w,b) inner); stride-2 reads
            rhs = AP(
                g.tensor,
                g.offset + i * REG + 16,
                [[BW, WP], [1, 2], [2, NB]],
            )
            nc.tensor.matmul(pt[h][:], lhsT, rhs, start=(i == 0), stop=(i == 4))

        # scatter 8 rows ahead: job r+8's buffer was last read by job r-4,
        # whose final matmul is in this very row, so the WAR wait is
        # satisfied by the time the DMA queue reaches it
        if r + 8 < NH:
            scatter(r + 8)

        if r >= 4:
            j = r - 4  # job whose accumulation just finished
            if j % 8 == 0:
                ob = obp.tile([C, 8 * 2 * NB], BF16, name=f"ob{j // 8}", tag="ob")
            sl = ob[:, (j % 8) * 2 * NB : (j % 8 + 1) * 2 * NB]
            if j % 2 == 0:
                nc.vector.tensor_copy(sl, pt[j][:])
            else:
                nc.scalar.copy(sl, pt[j][:])
            del pt[j]
            if j == NH - 5:
                nc.gpsimd.dma_start(
                    out_ap[:, 2 * (NH - 8) : 2 * (NH - 4), :], ob[:, : 4 * 2 * NB]
                )
            elif j == NH - 3:
                nc.gpsimd.dma_start(
                    out_ap[:, 2 * (NH - 4) : 2 * (NH - 2), :],
                    ob[:, 4 * 2 * NB : 6 * 2 * NB],
                )
            elif j == NH - 1:
                nc.gpsimd.dma_start(
                    out_ap[:, 2 * (NH - 2) :, :], ob[:, 6 * 2 * NB :]
                )
            elif j % 8 == 7:
                q = j // 8
                nc.gpsimd.dma_start(out_ap[:, 16 * q : 16 * q + 16, :], ob[:])


def _build_program():
    nc = bacc.Bacc(
        "TRN2", debug=False, enable_asserts=False, target_bir_lowering=False
    )
    ft_t = nc.dram_tensor("ft", [WP, NROWS * C], BF16, kind="ExternalInput")
    msk_t = nc.dram_tensor("msk3", [WP, MSK_COLS], BF16, kind="ExternalInput")
    out_t = nc.dram_tensor("out", [C, 2 * NH, NB], BF16, kind="ExternalOutput")

    with tile.TileContext(nc) as tc, ExitStack() as ctx:
        _device_body(tc, ctx, out_t.ap(), ft_t.ap(), msk_t.ap())
    nc.compile()
    return nc


def _prep_ft(feat_n: np.ndarray, h0: int) -> np.ndarray:
    """[C,H,W] -> fT[w'', r, c] with r over [h0-2, h0+NH+2), zero-padded."""
    ft = np.zeros((WP, NROWS, C), np.float32)
    r_lo, r_hi = h0 - 2, h0 + NH + 2
    s_lo, s_hi = max(r_lo, 0), min(r_hi, H)
    ft[PAD : PAD + W, s_lo - r_lo : s_hi - r_lo, :] = feat_n[:, s_lo:s_hi, :].transpose(
        2, 1, 0
    )
    return ft.reshape(WP, NROWS * C).astype(ml_dtypes.bfloat16)


def _prep_msk3(masks_n: np.ndarray) -> np.ndarray:
    """[25, 2H, 2W] -> msk3[w', h, i, t20]  [WP, H, KS, RUN]
    t20 = (w - (w'-4))*4 + b*2 + a; value = masks[5i + (4 - t20//4), 2h+a, 2w+b]
    """
    tt = np.arange(RUN)
    wpp = np.arange(WP)
    dw = tt // 4
    b = (tt % 4) // 2
    a = tt % 2
    j = 4 - dw
    wup = 2 * (wpp[:, None] - 4 + dw[None, :]) + b[None, :]
    wup_c = np.clip(wup, 0, 2 * W - 1)                     # [WP, RUN]
    i_ar = np.arange(KS)
    k_full = 5 * i_ar[:, None] + j[None, :]                # [KS, RUN]
    hh = np.arange(H)
    hup = 2 * hh[:, None] + a[None, :]                     # [H, RUN]
    out = masks_n[
        k_full[None, None, :, :],
        hup[None, :, None, :],
        wup_c[:, None, None, :],
    ]  # [WP, H, KS, RUN]
    return out.astype(np.float32)


def kernel(features: np.ndarray, masks: np.ndarray, _perf: dict | None = None):
    features = np.asarray(features, dtype=np.float32)
    masks = np.asarray(masks, dtype=np.float32)

    if "nc" not in _PROG_CACHE:
        _PROG_CACHE["nc"] = _build_program()
    nc = _PROG_CACHE["nc"]

    in_maps = []
    for core in range(8):
        n, half = divmod(core, 2)
        h0 = NH * half
        ft_sh = _prep_ft(features[n], h0)
        msk3 = _prep_msk3(masks[n])[:, h0 : h0 + NH]  # [WP, NH, KS, RUN]
        in_maps.append(
            {
                "ft": ft_sh,
                "msk3": np.ascontiguousarray(
                    msk3.reshape(WP, MSK_COLS)
                ).astype(ml_dtypes.bfloat16),
            }
        )

    trace = bool(_perf is not None and _perf.get("trace"))
    res = run_bass_kernel_spmd(
        nc, in_maps, core_ids=list(range(8)), trace=trace,
        **({} if not trace else {"trace_cores": [0]}),
    )
    if _perf is not None:
        _perf["exec_time_ns"] = res.exec_time_ns
        _perf["trace"] = res.instructions_and_trace

    out = np.empty((N, C, SCALE * H, SCALE * W), np.float32)
    for core in range(8):
        n, half = divmod(core, 2)
        out[n, :, 64 * half : 64 * half + 64, :] = np.asarray(
            res.results[core]["out"]
        ).astype(np.float32)
    return out


# revision 17
# speedup vs baseline: 1.1373x; 1.1373x over previous
"""CARAFE content-aware upsampling on 8 Trainium2 NeuronCores (Bass/Tile).

Problem: features (4,128,64,64) f32, masks (4,25,128,128) f32
         -> out (4,128,128,128) f32
out[n,c,2h+a,2w+b] = sum_{i,j in 5x5} f[n,c,h+i-2,w+j-2] * m[n,5i+j,2h+a,2w+b]

v6 strategy (per core = one (n, h-half) shard), all-bf16 on device:
  Weight-stationary over shard feature rows r (36 of them): one bf16
  LDWEIGHTS of fT_row(r) [w''(68), c(128)] feeds up to 5 accumulating
  matmuls, one per in-flight job h = r-4..r (i = r-h), each into its own
  PSUM tile:  out_job(h) [c, (a,wup)=256] += fT_row(h+i).T @ B_i(h).
  B_i is the banded mask matrix [68, 256-in-288-region], materialized per
  job by an SBUF->SBUF diagonal-scatter DMA (340 packets of 40 B) into
  one of 12 rotating per-job band buffers.  Each HWDGE ring drains ~0.235
  packets/ns, so per-job scatters alternate sync/scalar (~1.45 us each,
  ~23 us/queue total, hidden under the ~34 us matmul stream); scatters
  are emitted 6 rows ahead so their WAR waits are pre-satisfied.  Band
  zeros are memset once (f32-bitcast halves the element count) split
  across vector+gpsimd; big-packet loads (features, late masks) ride
  gpsimd's 16-engine SWDGE queue.  PSUM f32 results are cast-copied to
  bf16 staging (vector/scalar alternating) and DMA'd out 8 jobs per
  store on gpsimd; the host upcasts to f32.  rel err ~3e-3 vs 2e-2 gate.
"""
import sys

if "/opt/trn_rl_repo" not in sys.path:
    sys.path.insert(0, "/opt/trn_rl_repo")

from contextlib import ExitStack

import ml_dtypes
import numpy as np

import concourse.tile as tile
from concourse import bacc, mybir
from concourse.ap import AP
from concourse.bass_utils import run_bass_kernel_spmd

# ---- problem constants (hardcoded per harness contract) ----
N = 4
C = 128
H = 64
W = 64
KS = 5
PAD = 2
SCALE = 2
WP = W + KS - 1          # 68 contraction width per feature row
NB = SCALE * W           # 128 upsampled cols per hup row
RUN = 4 * KS             # 20 elems per diagonal run (w,b,a interleaved)
REG = 2 * NB + 32        # 288 per-band region: 16 pad | 256 data | 16 pad
BW = KS * REG            # 1440 band elems per job
NH = H // 2              # 32 low-res rows (jobs) per core
NROWS = NH + 4           # 36 feature rows per shard (halo zero-padded)
NBUF = 12                # rotating per-job band buffers
MSK_COLS = NH * KS * RUN

F32 = mybir.dt.float32
BF16 = mybir.dt.bfloat16

_PROG_CACHE: dict = {}


def _device_body(tc, ctx, out_ap, ft_ap, msk3_ap):
    nc = tc.nc
    sb = ctx.enter_context(tc.tile_pool(name="sb", bufs=1))
    psum = ctx.enter_context(tc.tile_pool(name="ps", bufs=8, space="PSUM"))
    obp = ctx.enter_context(tc.tile_pool(name="ob", bufs=2))

    ft = sb.tile([WP, NROWS * C], BF16)
    mst = sb.tile([WP, MSK_COLS], BF16)
    bufs = [
        sb.tile([WP, BW], BF16, name=f"bb{k}", tag=f"bb{k}") for k in range(NBUF)
    ]

    # early masks on sync; features + late masks on gpsimd's 16-engine
    # SWDGE queue; band zero-fill (f32-bitcast) split vector/gpsimd
    nc.sync.dma_start(mst[:, : 8 * KS * RUN], msk3_ap[:, : 8 * KS * RUN])
    nc.gpsimd.dma_start(ft[:, : 6 * C], ft_ap[:, : 6 * C])
    nc.gpsimd.dma_start(mst[:, 8 * KS * RUN :], msk3_ap[:, 8 * KS * RUN :])
    for k in range(0, NBUF, 2):
        nc.vector.memset(bufs[k][:].bitcast(F32), 0.0)
    for k in (1, 3, 5):
        nc.gpsimd.memset(bufs[k][:].bitcast(F32), 0.0)
    nc.gpsimd.dma_start(ft[:, 6 * C : 18 * C], ft_ap[:, 6 * C : 18 * C])
    for k in (7, 9, 11):
        nc.gpsimd.memset(bufs[k][:].bitcast(F32), 0.0)
    nc.gpsimd.dma_start(ft[:, 18 * C :], ft_ap[:, 18 * C :])

    def scatter(j):
        # job j's 5 band regions: 340 diagonal 40 B runs, alternating rings
        g = bufs[j % NBUF][:]
        dst = AP(g.tensor, g.offset, [[BW + 4, WP], [REG, KS], [1, RUN]])
        m = mst[:]
        src = AP(
            m.tensor,
            m.offset + j * KS * RUN,
            [[MSK_COLS, WP], [RUN, KS], [1, RUN]],
        )
        eng = nc.sync if j % 2 == 0 else nc.scalar
        eng.dma_start(dst, src)

    for j in range(6):
        scatter(j)

    pt = {}
    ob = None
    for r in range(NROWS):
        lhsT = ft[:, r * C : (r + 1) * C]
        for h in range(max(0, r - 4), min(NH - 1, r) + 1):
            i = r - h
            if i == 0:
                pt[h] = psum.tile([C, 2 * NB], F32, name=f"pt{h}", tag="pt")
            g = bufs[h % NBUF][:]
            # psum col = a*NB + wup (a outer, (w,b) inner); stride-2 reads
            rhs = AP(
                g.tensor,
                g.offset + i * REG + 16,
                [[BW, WP], [1, 2], [2, NB]],
            )
            nc.tensor.matmul(pt[h][:], lhsT, rhs, start=(i == 0), stop=(i == 4))

        # scatter 6 rows ahead: job r+6's buffer was last read at row r-2,
        # so the WAR wait is already satisfied when the DMA queue reaches it
        if 6 <= r + 6 < NH:
            scatter(r + 6)

        if r >= 4:
            j = r - 4  # job whose accumulation just finished
            if j % 8 == 0:
                ob = obp.tile([C, 8 * 2 * NB], BF16, name=f"ob{j // 8}", tag="ob")
            sl = ob[:, (j % 8) * 2 * NB : (j % 8 + 1) * 2 * NB]
            if j % 2 == 0:
                nc.vector.tensor_copy(sl, pt[j][:])
            else:
                nc.scalar.copy(sl, pt[j][:])
            del pt[j]
            if j % 8 == 7:
                q = j // 8
                nc.gpsimd.dma_start(out_ap[:, 16 * q : 16 * q + 16, :], ob[:])


def _build_program():
    nc = bacc.Bacc(
        "TRN2", debug=False, enable_asserts=False, target_bir_lowering=False
    )
    ft_t = nc.dram_tensor("ft", [WP, NROWS * C], BF16, kind="ExternalInput")
    msk_t = nc.dram_tensor("msk3", [WP, MSK_COLS], BF16, kind="ExternalInput")
    out_t = nc.dram_tensor("out", [C, 2 * NH, NB], BF16, kind="ExternalOutput")

    with tile.TileContext(nc) as tc, ExitStack() as ctx:
        _device_body(tc, ctx, out_t.ap(), ft_t.ap(), msk_t.ap())
    nc.compile()
    return nc


def _prep_ft(feat_n: np.ndarray, h0: int) -> np.ndarray:
    """[C,H,W] -> fT[w'', r, c] with r over [h0-2, h0+NH+2), zero-padded."""
    ft = np.zeros((WP, NROWS, C), np.float32)
    r_lo, r_hi = h0 - 2, h0 + NH + 2
    s_lo, s_hi = max(r_lo, 0), min(r_hi, H)
    ft[PAD : PAD + W, s_lo - r_lo : s_hi - r_lo, :] = feat_n[:, s_lo:s_hi, :].transpose(
        2, 1, 0
    )
    return ft.reshape(WP, NROWS * C).astype(ml_dtypes.bfloat16)


def _prep_msk3(masks_n: np.ndarray) -> np.ndarray:
    """[25, 2H, 2W] -> msk3[w', h, i, t20]  [WP, H, KS, RUN]
    t20 = (w - (w'-4))*4 + b*2 + a; value = masks[5i + (4 - t20//4), 2h+a, 2w+b]
    """
    tt = np.arange(RUN)
    wpp = np.arange(WP)
    dw = tt // 4
    b = (tt % 4) // 2
    a = tt % 2
    j = 4 - dw
    wup = 2 * (wpp[:, None] - 4 + dw[None, :]) + b[None, :]
    wup_c = np.clip(wup, 0, 2 * W - 1)                     # [WP, RUN]
    i_ar = np.arange(KS)
    k_full = 5 * i_ar[:, None] + j[None, :]                # [KS, RUN]
    hh = np.arange(H)
    hup = 2 * hh[:, None] + a[None, :]                     # [H, RUN]
    out = masks_n[
        k_full[None, None, :, :],
        hup[None, :, None, :],
        wup_c[:, None, None, :],
    ]  # [WP, H, KS, RUN]
    return out.astype(np.float32)


def kernel(features: np.ndarray, masks: np.ndarray, _perf: dict | None = None):
    features = np.asarray(features, dtype=np.float32)
    masks = np.asarray(masks, dtype=np.float32)

    if "nc" not in _PROG_CACHE:
        _PROG_CACHE["nc"] = _build_program()
    nc = _PROG_CACHE["nc"]

    in_maps = []
    for core in range(8):
        n, half = divmod(core, 2)
        h0 = NH * half
        ft_sh = _prep_ft(features[n], h0)
        msk3 = _prep_msk3(masks[n])[:, h0 : h0 + NH]  # [WP, NH, KS, RUN]
        in_maps.append(
            {
                "ft": ft_sh,
                "msk3": np.ascontiguousarray(
                    msk3.reshape(WP, MSK_COLS)
                ).astype(ml_dtypes.bfloat16),
            }
        )

    trace = bool(_perf is not None and _perf.get("trace"))
    res = run_bass_kernel_spmd(
        nc, in_maps, core_ids=list(range(8)), trace=trace,
        **({} if not trace else {"trace_cores": [0]}),
    )
    if _perf is not None:
        _perf["exec_time_ns"] = res.exec_time_ns
        _perf["trace"] = res.instructions_and_trace

    out = np.empty((N, C, SCALE * H, SCALE * W), np.float32)
    for core in range(8):
        n, half = divmod(core, 2)
        out[n, :, 64 * half : 64 * half + 64, :] = np.asarray(
            res.results[core]["out"]
        ).astype(np.float32)
    return out


# revision 20
# speedup vs baseline: 1.1757x; 1.0337x over previous
"""CARAFE content-aware upsampling on 8 Trainium2 NeuronCores (Bass/Tile).

Problem: features (4,128,64,64) f32, masks (4,25,128,128) f32
         -> out (4,128,128,128) f32
out[n,c,2h+a,2w+b] = sum_{i,j in 5x5} f[n,c,h+i-2,w+j-2] * m[n,5i+j,2h+a,2w+b]

v6 strategy (per core = one (n, h-half) shard), all-bf16 on device:
  Weight-stationary over shard feature rows r (36 of them): one bf16
  LDWEIGHTS of fT_row(r) [w''(68), c(128)] feeds up to 5 accumulating
  matmuls, one per in-flight job h = r-4..r (i = r-h), each into its own
  PSUM tile:  out_job(h) [c, (a,wup)=256] += fT_row(h+i).T @ B_i(h).
  B_i is the banded mask matrix [68, 256-in-288-region], materialized per
  job by an SBUF->SBUF diagonal-scatter DMA (340 packets of 40 B) into
  one of 12 rotating per-job band buffers.  Each HWDGE ring drains ~0.235
  packets/ns, so per-job scatters alternate sync/scalar (~1.45 us each,
  ~23 us/queue total, hidden under the ~34 us matmul stream); scatters
  are emitted 6 rows ahead so their WAR waits are pre-satisfied.  Band
  zeros are memset once (f32-bitcast halves the element count) split
  across vector+gpsimd; big-packet loads (features, late masks) ride
  gpsimd's 16-engine SWDGE queue.  PSUM f32 results are cast-copied to
  bf16 staging (vector/scalar alternating) and DMA'd out 8 jobs per
  store on gpsimd; the host upcasts to f32.  rel err ~3e-3 vs 2e-2 gate.
"""
import sys

if "/opt/trn_rl_repo" not in sys.path:
    sys.path.insert(0, "/opt/trn_rl_repo")

from contextlib import ExitStack

import ml_dtypes
import numpy as np

import concourse.tile as tile
from concourse import bacc, mybir
from concourse.ap import AP
from concourse.bass_utils import run_bass_kernel_spmd

# ---- problem constants (hardcoded per harness contract) ----
N = 4
C = 128
H = 64
W = 64
KS = 5
PAD = 2
SCALE = 2
WP = W + KS - 1          # 68 contraction width per feature row
NB = SCALE * W           # 128 upsampled cols per hup row
RUN = 4 * KS             # 20 elems per diagonal run (w,b,a interleaved)
REG = 2 * NB + 32        # 288 per-band region: 16 pad | 256 data | 16 pad
BW = KS * REG            # 1440 band elems per job
NH = H // 2              # 32 low-res rows (jobs) per core
NROWS = NH + 4           # 36 feature rows per shard (halo zero-padded)
NBUF = 12                # rotating per-job band buffers
MSK_COLS = NH * KS * RUN

F32 = mybir.dt.float32
BF16 = mybir.dt.bfloat16

_PROG_CACHE: dict = {}


def _device_body(tc, ctx, out_ap, ft_ap, msk3_ap):
    nc = tc.nc
    sb = ctx.enter_context(tc.tile_pool(name="sb", bufs=1))
    psum = ctx.enter_context(tc.tile_pool(name="ps", bufs=8, space="PSUM"))
    obp = ctx.enter_context(tc.tile_pool(name="ob", bufs=2))

    ft = sb.tile([WP, NROWS * C], BF16)
    mst = sb.tile([WP, MSK_COLS], BF16)
    bufs = [
        sb.tile([WP, BW], BF16, name=f"bb{k}", tag=f"bb{k}") for k in range(NBUF)
    ]

    # early masks on sync; features + late masks on gpsimd's 16-engine
    # SWDGE queue; band zero-fill (f32-bitcast) split vector/gpsimd
    nc.sync.dma_start(mst[:, : 8 * KS * RUN], msk3_ap[:, : 8 * KS * RUN])
    nc.gpsimd.dma_start(ft[:, : 6 * C], ft_ap[:, : 6 * C])
    nc.gpsimd.dma_start(mst[:, 8 * KS * RUN :], msk3_ap[:, 8 * KS * RUN :])
    for k in range(0, NBUF, 2):
        nc.vector.memset(bufs[k][:].bitcast(F32), 0.0)
    for k in (1, 3, 5):
        nc.gpsimd.memset(bufs[k][:].bitcast(F32), 0.0)
    nc.gpsimd.dma_start(ft[:, 6 * C : 18 * C], ft_ap[:, 6 * C : 18 * C])
    for k in (7, 9, 11):
        nc.gpsimd.memset(bufs[k][:].bitcast(F32), 0.0)
    nc.gpsimd.dma_start(ft[:, 18 * C :], ft_ap[:, 18 * C :])

    def scatter(j):
        # job j's 5 band regions: 340 diagonal 40 B runs, alternating rings
        g = bufs[j % NBUF][:]
        dst = AP(g.tensor, g.offset, [[BW + 4, WP], [REG, KS], [1, RUN]])
        m = mst[:]
        src = AP(
            m.tensor,
            m.offset + j * KS * RUN,
            [[MSK_COLS, WP], [RUN, KS], [1, RUN]],
        )
        eng = nc.sync if j % 2 == 0 else nc.scalar
        eng.dma_start(dst, src)

    for j in range(6):
        scatter(j)

    pt = {}
    ob = None
    for r in range(NROWS):
        lhsT = ft[:, r * C : (r + 1) * C]
        for h in range(max(0, r - 4), min(NH - 1, r) + 1):
            i = r - h
            if i == 0:
                pt[h] = psum.tile([C, 2 * NB], F32, name=f"pt{h}", tag="pt")
            g = bufs[h % NBUF][:]
            # psum col = a*NB + wup (a outer, (w,b) inner); stride-2 reads
            rhs = AP(
                g.tensor,
                g.offset + i * REG + 16,
                [[BW, WP], [1, 2], [2, NB]],
            )
            nc.tensor.matmul(pt[h][:], lhsT, rhs, start=(i == 0), stop=(i == 4))

        # scatter 6 rows ahead: job r+6's buffer was last read at row r-2,
        # so the WAR wait is already satisfied when the DMA queue reaches it
        if 6 <= r + 6 < NH:
            scatter(r + 6)

        if r >= 4:
            j = r - 4  # job whose accumulation just finished
            if j % 8 == 0:
                ob = obp.tile([C, 8 * 2 * NB], BF16, name=f"ob{j // 8}", tag="ob")
            sl = ob[:, (j % 8) * 2 * NB : (j % 8 + 1) * 2 * NB]
            if j % 2 == 0:
                nc.vector.tensor_copy(sl, pt[j][:])
            else:
                nc.scalar.copy(sl, pt[j][:])
            del pt[j]
            if j % 8 == 7:
                q = j // 8
                nc.gpsimd.dma_start(out_ap[:, 16 * q : 16 * q + 16, :], ob[:])


def _build_program():
    nc = bacc.Bacc(
        "TRN2", debug=False, enable_asserts=False, target_bir_lowering=False
    )
    ft_t = nc.dram_tensor("ft", [WP, NROWS * C], BF16, kind="ExternalInput")
    msk_t = nc.dram_tensor("msk3", [WP, MSK_COLS], BF16, kind="ExternalInput")
    out_t = nc.dram_tensor("out", [C, 2 * NH, NB], BF16, kind="ExternalOutput")

    with tile.TileContext(nc) as tc, ExitStack() as ctx:
        _device_body(tc, ctx, out_t.ap(), ft_t.ap(), msk_t.ap())
    nc.compile()
    return nc


def _prep_ft(feat_n: np.ndarray, h0: int) -> np.ndarray:
    """[C,H,W] -> fT[w'', r, c] with r over [h0-2, h0+NH+2), zero-padded."""
    ft = np.zeros((WP, NROWS, C), np.float32)
    r_lo, r_hi = h0 - 2, h0 + NH + 2
    s_lo, s_hi = max(r_lo, 0), min(r_hi, H)
    ft[PAD : PAD + W, s_lo - r_lo : s_hi - r_lo, :] = feat_n[:, s_lo:s_hi, :].transpose(
        2, 1, 0
    )
    return ft.reshape(WP, NROWS * C).astype(ml_dtypes.bfloat16)


def _prep_msk3(masks_n: np.ndarray) -> np.ndarray:
    """[25, 2H, 2W] -> msk3[w', h, i, t20]  [WP, H, KS, RUN]
    t20 = (w - (w'-4))*4 + b*2 + a; value = masks[5i + (4 - t20//4), 2h+a, 2w+b]
    """
    tt = np.arange(RUN)
    wpp = np.arange(WP)
    dw = tt // 4
    b = (tt % 4) // 2
    a = tt % 2
    j = 4 - dw
    wup = 2 * (wpp[:, None] - 4 + dw[None, :]) + b[None, :]
    wup_c = np.clip(wup, 0, 2 * W - 1)                     # [WP, RUN]
    i_ar = np.arange(KS)
    k_full = 5 * i_ar[:, None] + j[None, :]                # [KS, RUN]
    hh = np.arange(H)
    hup = 2 * hh[:, None] + a[None, :]                     # [H, RUN]
    out = masks_n[
        k_full[None, None, :, :],
        hup[None, :, None, :],
        wup_c[:, None, None, :],
    ]  # [WP, H, KS, RUN]
    return out.astype(np.float32)


def kernel(features: np.ndarray, masks: np.ndarray, _perf: dict | None = None):
    features = np.asarray(features, dtype=np.float32)
    masks = np.asarray(masks, dtype=np.float32)

    if "nc" not in _PROG_CACHE:
        _PROG_CACHE["nc"] = _build_program()
    nc = _PROG_CACHE["nc"]

    in_maps = []
    for core in range(8):
        n, half = divmod(core, 2)
        h0 = NH * half
        ft_sh = _prep_ft(features[n], h0)
        msk3 = _prep_msk3(masks[n])[:, h0 : h0 + NH]  # [WP, NH, KS, RUN]
        in_maps.append(
            {
                "ft": ft_sh,
                "msk3": np.ascontiguousarray(
                    msk3.reshape(WP, MSK_COLS)
                ).astype(ml_dtypes.bfloat16),
            }
        )

    trace = bool(_perf is not None and _perf.get("trace"))
    res = run_bass_kernel_spmd(
        nc, in_maps, core_ids=list(range(8)), trace=trace,
        **({} if not trace else {"trace_cores": [0]}),
    )
    if _perf is not None:
        _perf["exec_time_ns"] = res.exec_time_ns
        _perf["trace"] = res.instructions_and_trace

    out = np.empty((N, C, SCALE * H, SCALE * W), np.float32)
    for core in range(8):
        n, half = divmod(core, 2)
        out[n, :, 64 * half : 64 * half + 64, :] = np.asarray(
            res.results[core]["out"]
        ).astype(np.float32)
    return out
